# revision 1
# baseline (speedup 1.0000x reference)
"""CommNet forward kernel for 8 Trainium2 NeuronCores.

Reference computation (per sample of N=32 agents, batch B=16384):
    h   = relu(obs @ enc_w + enc_b)                    # [B,N,64]
    2x:  msg = (sum_n h - h)/31
         h   = relu(concat(h, msg) @ comm_w[r] + comm_b[r])
    hid = relu(h @ out_w1 + out_b1)
    q   = hid @ out_w2 + out_b2; q[avail==0] = -1e10

Device strategy (pure data parallel, batch split 8 ways):
  * activations feature-major [feat(part), row(free)]; four 512-row
    groups per 2048-row super-iteration, packed 2x2 into the PE array
    via tile_position (K=64, M=64 quadrants) so all 16 subarrays
    compute concurrently.  Groups at (p-half, f-half) positions
    (0,1)/(1,0) swap every matmul layer; 4 permuting layers = identity,
    so the out2 layout matches the obs layout.
  * comm round rewritten as h @ W_self + S @ W_sum with
    W_self = W_h - W_m/31, W_sum = W_m/31, S = per-sample agent sum.
    S comes from identity-weight matmuls with a step-0 (broadcast)
    output AP that accumulates the 32 agent columns of each sample into
    one PSUM column; the S @ W_sum term re-broadcasts S via a step-0
    rhs AP into the same accumulation group as the W_self matmul.
  * relu+bias fused into the PSUM->SBUF evacuation (DVE dual-op
    tensor_scalar for enc/out1, ScalarE activation for the rounds)
  * mask+final bias folded host-side into pen = where(avail, out_b2, -1e10);
    pen is added on the PE (identity-lhsT matmul accumulate) and the q
    bank evacuated with a ScalarE copy
  * host pre-packs obs into the feature-major layout and unpacks q
    (layout work is free on host; the device does all the FLOPs)
"""

import contextlib
import sys

import numpy as np

sys.path.insert(0, "/opt/trn_rl_repo")

import ml_dtypes  # noqa: E402

B, N, OBS, H, A, NR = 16384, 32, 64, 64, 16, 2
NCORES = 8
RPC = B * N // NCORES   # rows per core = 65536

SUP = 2048              # rows per super-iteration (4 groups of 512)
GRP = 512               # rows per group (one fp32 PSUM bank)
NSUP = RPC // SUP
NS_G = GRP // N         # samples per group = 16
NS_H = 2 * NS_G         # samples per partition-half per super = 32

_cache = {}


def _build_device_program():
    import concourse.bacc as bacc
    import concourse.mybir as mybir
    from concourse import tile

    F32 = mybir.dt.float32
    BF16 = mybir.dt.bfloat16

    nc = bacc.Bacc("TRN2", target_bir_lowering=False, debug=False)

    obs_d = nc.dram_tensor("obs_pk", [NSUP, 128, SUP // 2], BF16, kind="ExternalInput")
    pen_d = nc.dram_tensor("pen_pk", [NSUP // 2, 128, GRP], F32, kind="ExternalInput")
    q_d = nc.dram_tensor("q_pk", [NSUP // 2, 128, GRP], BF16, kind="ExternalOutput")

    # replicated-on-both-halves [128, 64] weights; W2 block-diag [128, 32]
    wname = ["Wenc", "Wself0", "Wself1", "Wsum0", "Wsum1", "W1", "idn"]
    w_d = {n: nc.dram_tensor(n, [128, 64], BF16, kind="ExternalInput") for n in wname}
    w_d["W2"] = nc.dram_tensor("W2", [128, 32], BF16, kind="ExternalInput")
    w_d["idnq"] = nc.dram_tensor("idnq", [128, 32], F32, kind="ExternalInput")
    bname = ["be", "b0", "b1", "bh"]
    b_d = {n: nc.dram_tensor(n, [128, 1], F32, kind="ExternalInput") for n in bname}

    FD = GRP
    Relu = mybir.ActivationFunctionType.Relu
    Copy = mybir.ActivationFunctionType.Copy
    ALU = mybir.AluOpType
    QUAD = [(0, 0, 0, 0), (0, 1, 0, 64), (1, 0, 64, 64), (1, 1, 64, 0)]
    # (in p-half, in f-half, rhs part base, out part base); out f-half = in f-half
    # after act: group at (ph, fh) lands at (out_base//64, fh) -> (0,1)/(1,0) swap

    with tile.TileContext(nc) as tc, contextlib.ExitStack() as ctx:
        wp = ctx.enter_context(tc.tile_pool(name="w", bufs=1))
        pool = ctx.enter_context(tc.tile_pool(name="p", bufs=3))
        psum = ctx.enter_context(tc.tile_pool(name="ps", bufs=1, space="PSUM"))

        W = {}
        for n in wname:
            W[n] = wp.tile([128, 64], BF16, tag=n, name=f"w_{n}")
            nc.sync.dma_start(W[n][:], w_d[n][:])
        W["W2"] = wp.tile([128, 32], BF16, tag="W2", name="w_W2")
        nc.sync.dma_start(W["W2"][:], w_d["W2"][:])
        W["idnq"] = wp.tile([128, 32], F32, tag="idnq", name="w_idnq")
        nc.sync.dma_start(W["idnq"][:], w_d["idnq"][:])
        BIAS = {}
        for n in bname:
            BIAS[n] = wp.tile([128, 1], F32, tag=n, name=f"b_{n}")
            nc.sync.dma_start(BIAS[n][:], b_d[n][:])

        def layer_mms(ps, wt, rhs_t):
            """4 concurrent K=64/M=64 matmuls (one per group) into ps[128,1024]."""
            for ph, fh, rb, ob in QUAD:
                nc.tensor.matmul(
                    ps[ob:ob + 64, fh * FD:(fh + 1) * FD],
                    wt[rb:rb + 64, :],
                    rhs_t[ph * 64:(ph + 1) * 64, fh * FD:(fh + 1) * FD],
                    start=True, stop=True, tile_position=(rb, ob),
                )

        for s in range(NSUP):
            obs_t = pool.tile([128, 2 * FD], BF16, tag="obs")
            nc.sync.dma_start(obs_t[:], obs_d[s])

            psE = psum.tile([128, 2 * FD], F32, tag="stg", bufs=3)
            layer_mms(psE, W["Wenc"], obs_t)
            h = pool.tile([128, 2 * FD], BF16, tag="h0")
            nc.vector.tensor_scalar(h[:], psE[:], BIAS["be"][:], 0.0,
                                    ALU.add, ALU.max)

            for r in range(NR):
                psS = psum.tile([128, NS_H], F32, tag="S")
                for hp, tp in ((0, 0), (64, 64)):
                    for sh in range(2):
                        rhs = h[hp:hp + 64, sh * FD:(sh + 1) * FD] \
                            .rearrange("p (S n) -> p n S", n=N)
                        outS = psS[hp:hp + 64, sh * NS_G:(sh + 1) * NS_G] \
                            .unsqueeze(1).broadcast_to([64, N, NS_G])
                        nc.tensor.matmul(outS, W["idn"][hp:hp + 64, :], rhs,
                                         start=True, stop=True,
                                         tile_position=(tp, tp))
                S2 = pool.tile([128, NS_H], BF16, tag="S2")
                nc.vector.tensor_copy(S2[:], psS[:])

                psR = psum.tile([128, 2 * FD], F32, tag="stg", bufs=3)
                for ph, fh, rb, ob in QUAD:
                    nc.tensor.matmul(
                        psR[ob:ob + 64, fh * FD:(fh + 1) * FD],
                        W[f"Wself{r}"][rb:rb + 64, :],
                        h[ph * 64:(ph + 1) * 64, fh * FD:(fh + 1) * FD],
                        start=True, stop=False, tile_position=(rb, ob),
                    )
                    sb = S2[ph * 64:(ph + 1) * 64, fh * NS_G:(fh + 1) * NS_G] \
                        .unsqueeze(2).broadcast_to([64, NS_G, N])
                    nc.tensor.matmul(
                        psR[ob:ob + 64, fh * FD:(fh + 1) * FD],
                        W[f"Wsum{r}"][rb:rb + 64, :], sb,
                        start=False, stop=True, tile_position=(rb, ob),
                    )
                h = pool.tile([128, 2 * FD], BF16, tag=f"h{1 + r}")
                nc.scalar.activation(h[:], psR[:], Relu, bias=BIAS[f"b{r}"][:])

            psH = psum.tile([128, 2 * FD], F32, tag="stg", bufs=3)
            layer_mms(psH, W["W1"], h)
            hid = pool.tile([128, 2 * FD], BF16, tag="hid")
            nc.vector.tensor_scalar(hid[:], psH[:], BIAS["bh"][:], 0.0,
                                    ALU.add, ALU.max)

            # out2: block-diag over partition pairs; two col positions.
            # q banks of even/odd super-iters pack into one [128, FD] bank
            # (partition halves) so the evacuation runs full-width half as often.
            k = s % 2
            qo = 64 * k
            if k == 0:
                pen_t = pool.tile([128, FD], F32, tag="pen")
                nc.sync.dma_start(pen_t[:], pen_d[s // 2])
                psQ = psum.tile([128, FD], F32, tag="q")
                pers = (pen_t, psQ)
            else:
                pen_t, psQ = pers
            nc.tensor.matmul(psQ[qo:qo + 32, :], W["W2"][:], hid[:, 0:FD],
                             start=True, stop=False, tile_position=(0, qo),
                             skip_group_check=True)
            nc.tensor.matmul(psQ[qo:qo + 32, :], W["idnq"][qo:qo + 32, :],
                             pen_t[qo:qo + 32, :],
                             start=False, stop=True, tile_position=(qo % 128 // 32 * 32, qo),
                             skip_group_check=True)
            nc.tensor.matmul(psQ[qo + 32:qo + 64, :], W["W2"][:], hid[:, FD:2 * FD],
                             start=True, stop=False, tile_position=(0, qo + 32),
                             skip_group_check=True)
            nc.tensor.matmul(psQ[qo + 32:qo + 64, :], W["idnq"][qo + 32:qo + 64, :],
                             pen_t[qo + 32:qo + 64, :],
                             start=False, stop=True,
                             tile_position=((qo + 32) % 128 // 32 * 32, qo + 32),
                             skip_group_check=True)
            if k == 1:
                q_sb = pool.tile([128, FD], BF16, tag="qsb")
                nc.scalar.activation(q_sb[:], psQ[:], Copy)
                nc.sync.dma_start(q_d[s // 2], q_sb[:])

    nc.compile()
    return nc


def _prep_host(obs, enc_w, enc_b, comm_w, comm_b, out_w1, out_b1, out_w2, out_b2,
               available_actions):
    """Build per-core input maps (packed layouts + derived weights)."""
    bf16 = ml_dtypes.bfloat16
    f32 = np.float32

    def rep(w):  # replicate [64, m] weight onto both partition halves
        return np.ascontiguousarray(np.concatenate([w, w], axis=0)
                                    .astype(f32)).astype(bf16)

    def bd(w):  # block-diag duplicate [k,m] -> [2k, 2m]
        k, m = w.shape
        o = np.zeros((2 * k, 2 * m), f32)
        o[:k, :m] = w
        o[k:, m:] = w
        return np.ascontiguousarray(o).astype(bf16)

    weights = {"Wenc": rep(enc_w), "W1": rep(out_w1), "W2": bd(out_w2),
               "idn": rep(np.eye(64, dtype=f32)),
               "idnq": np.ascontiguousarray(np.tile(np.eye(32, dtype=f32), (4, 1)))}
    for r in range(NR):
        wh = comm_w[r][:H].astype(f32)
        wm = comm_w[r][H:].astype(f32) / (N - 1)
        weights[f"Wself{r}"] = rep(wh - wm)
        weights[f"Wsum{r}"] = rep(wm)
    biases = {"be": enc_b, "b0": comm_b[0], "b1": comm_b[1], "bh": out_b1}
    biases = {k: np.concatenate([v, v]).astype(f32).reshape(128, 1)
              for k, v in biases.items()}

    rows = np.ascontiguousarray(obs.reshape(B * N, OBS))
    pen = np.where(available_actions.reshape(B * N, A) == 0,
                   f32(-1e10), out_b2.astype(f32)[None, :]).astype(f32)

    in_maps = []
    for c in range(NCORES):
        ro = rows[c * RPC:(c + 1) * RPC]
        # [NSUP, phalf, fhalf, row, feat] -> [NSUP, phalf*feat, fhalf*row]
        opk = ro.reshape(NSUP, 2, 2, GRP, OBS).transpose(0, 1, 4, 2, 3) \
                .reshape(NSUP, 128, SUP // 2).astype(bf16)
        pe = pen[c * RPC:(c + 1) * RPC]
        # q/pen partitions: [fhalf, phalf, action]
        ppk = pe.reshape(NSUP, 2, 2, GRP, A).transpose(0, 2, 1, 4, 3) \
                .reshape(NSUP // 2, 128, GRP).astype(f32)
        m = {"obs_pk": np.ascontiguousarray(opk),
             "pen_pk": np.ascontiguousarray(ppk)}
        m.update(weights)
        m.update(biases)
        in_maps.append(m)
    return in_maps


def _unpack_output(results):
    qs = []
    for r in results:
        qpk = np.asarray(r["q_pk"]).astype(np.float32)  # [NSUP//2, 128, GRP]
        q = qpk.reshape(NSUP, 2, 2, A, GRP).transpose(0, 2, 1, 4, 3) \
               .reshape(RPC, A)
        qs.append(q)
    return np.concatenate(qs, axis=0).reshape(B, N, A)


def run_on_device(in_maps, trace=False):
    from concourse.bass_utils import run_bass_kernel_spmd

    if "nc" not in _cache:
        _cache["nc"] = _build_device_program()
    return run_bass_kernel_spmd(_cache["nc"], in_maps,
                                core_ids=list(range(NCORES)), trace=trace)


def kernel(obs, enc_w, enc_b, comm_w, comm_b, out_w1, out_b1, out_w2, out_b2,
           available_actions):
    args = [np.asarray(x) for x in
            (obs, enc_w, enc_b, comm_w, comm_b, out_w1, out_b1, out_w2, out_b2,
             available_actions)]
    in_maps = _prep_host(*args)
    res = run_on_device(in_maps)
    return _unpack_output(res.results)



# revision 7
# speedup vs baseline: 2.3044x; 2.3044x over previous
"""CommNet forward kernel for 8 Trainium2 NeuronCores.

Reference computation (per sample of N=32 agents, batch B=16384):
    h   = relu(obs @ enc_w + enc_b)                    # [B,N,64]
    2x:  msg = (sum_n h - h)/31
         h   = relu(concat(h, msg) @ comm_w[r] + comm_b[r])
    hid = relu(h @ out_w1 + out_b1)
    q   = hid @ out_w2 + out_b2; q[avail==0] = -1e10

Device strategy (pure data parallel, batch split 8 ways):
  * comm round rewritten as h @ W_self + S @ W_sum with
    W_self = W_h - W_m/31, W_sum = W_m/31, S = per-sample agent sum.
  * iteration unit = 1024 rows (2 groups of 512, feature-major
    [128 = 2x64 feats, 512 rows]); 64 iterations per core.
  * the S-term is produced per round as a per-sample vector T and added
    into the 32 agent columns of each sample.  Round 0 uses the PE for
    T (agent-accumulating broadcast-output matmul of -W_sum, bias
    folded via K=1 rank-1 matmuls) and a max/add identity
    relu(x+t) = max(x,-t') + t'  split across DVE (max, PSUM read) and
    GPSIMD (add, SBUF only).  Round 1 computes S on the DVE (segmented
    reduce over agents), T with a tiny PE matmul, preloads T into PSUM
    on the scalar engine and lets the self-matmul accumulate on top,
    so its evacuation is a plain Relu.
  * mask+final bias folded host-side into pen = where(avail, out_b2,
    -1e10); added during the q-bank evacuation on the DVE.
  * work is spread so that per-iteration busy time is roughly balanced
    across PE / Act / DVE / Pool instead of serializing on the PE.
"""

import contextlib
import sys

import numpy as np

sys.path.insert(0, "/opt/trn_rl_repo")

import ml_dtypes  # noqa: E402

B, N, OBS, H, A, NR = 16384, 32, 64, 64, 16, 2
NCORES = 8
RPC = B * N // NCORES   # rows per core = 65536

ROWS = 1024             # rows per iteration (2 groups of 512)
GRP = 512               # rows per group
IT = RPC // ROWS        # iterations per core = 64
NS = GRP // N           # samples per group = 16

_cache = {}


def _build_device_program():
    import concourse.bacc as bacc
    import concourse.mybir as mybir
    from concourse import tile

    F32 = mybir.dt.float32
    BF16 = mybir.dt.bfloat16

    nc = bacc.Bacc("TRN2", target_bir_lowering=False, debug=False)

    obs_d = nc.dram_tensor("obs_pk", [IT, 128, GRP], BF16, kind="ExternalInput")
    pen_d = nc.dram_tensor("pen_pk", [IT // 4, 128, GRP], BF16, kind="ExternalInput")
    q_d = nc.dram_tensor("q_pk", [IT // 4, 128, GRP], BF16, kind="ExternalOutput")

    wname = ["Wenc", "Wself0", "Wself1", "Wsum0", "Wsum1", "W1", "bm0", "bm1"]
    w_d = {n: nc.dram_tensor(n, [128, 64], BF16, kind="ExternalInput") for n in wname}
    w_d["W2"] = nc.dram_tensor("W2", [128, 32], BF16, kind="ExternalInput")
    bname = ["be", "bh"]
    b_d = {n: nc.dram_tensor(n, [128, 1], F32, kind="ExternalInput") for n in bname}

    Relu = mybir.ActivationFunctionType.Relu
    Copy = mybir.ActivationFunctionType.Copy
    ALU = mybir.AluOpType

    with tile.TileContext(nc) as tc, contextlib.ExitStack() as ctx:
        wp = ctx.enter_context(tc.tile_pool(name="w", bufs=1))
        pool = ctx.enter_context(tc.tile_pool(name="p", bufs=2))
        psum = ctx.enter_context(tc.tile_pool(name="ps", bufs=1, space="PSUM"))

        W = {}
        for n in wname:
            W[n] = wp.tile([128, 64], BF16, tag=n, name=f"w_{n}")
            nc.sync.dma_start(W[n][:], w_d[n][:])
        W["W2"] = wp.tile([128, 32], BF16, tag="W2", name="w_W2")
        nc.sync.dma_start(W["W2"][:], w_d["W2"][:])
        BIAS = {}
        for n in bname:
            BIAS[n] = wp.tile([128, 1], F32, tag=n, name=f"b_{n}")
            nc.sync.dma_start(BIAS[n][:], b_d[n][:])
        ones16 = wp.tile([128, 16], BF16, tag="ones16", name="ones16")
        nc.vector.memset(ones16[:], 1.0)

        def big(tag, t):
            return psum.tile([128, GRP], F32, tag="big", bufs=5,
                             name=f"ps{tag}{t}")

        def layer_mm(wt, h_t, ps, start=True):
            for g in range(2):
                sl = slice(64 * g, 64 * g + 64)
                nc.tensor.matmul(ps[sl, :], W[wt][sl, :], h_t[sl, :],
                                 start=start, stop=True,
                                 skip_group_check=not start)
            return ps

        # per-iteration live state, keyed by iteration index
        st = {u: {} for u in range(IT)}
        LAG = 6

        # prologue DMAs + enc(0)
        for u in (0, 1):
            o = pool.tile([128, GRP], BF16, tag="obs", bufs=3, name=f"obs{u}")
            nc.sync.dma_start(o[:], obs_d[u])
            st[u]["obs"] = o
        st[0]["psE"] = layer_mm("Wenc", st[0]["obs"], big("E", 0))

        for s in range(IT + LAG):
            # [SP] prefetch obs(s+2)
            if s + 2 < IT:
                o = pool.tile([128, GRP], BF16, tag="obs", bufs=3,
                              name=f"obs{s + 2}")
                nc.sync.dma_start(o[:], obs_d[s + 2])
                st[s + 2]["obs"] = o
            # [SP] prefetch pen two bodies ahead of its quad's q-evac
            uq = s - 3
            if 0 <= uq < IT and uq % 4 == 0:
                pen_t = pool.tile([128, GRP], BF16, tag="pen", bufs=2,
                                  name=f"pen{uq}")
                nc.sync.dma_start(pen_t[:], pen_d[uq // 4])
                st[uq]["pen"] = pen_t

            # [PE] enc(s+1)
            if s + 1 < IT:
                st[s + 1]["psE"] = layer_mm("Wenc", st[s + 1]["obs"],
                                            big("E", s + 1))
                del st[s + 1]["obs"]

            # [Act] preload T0(s-1) into psR0; [PE] s0(s-1)
            u = s - 1
            if 0 <= u < IT:
                psR0 = big("R0", u)
                nc.scalar.activation(
                    psR0[:].rearrange("p (S n) -> p S n", n=N),
                    st[u]["psT0"][:].unsqueeze(2).broadcast_to([128, NS, N]),
                    Copy)
                del st[u]["psT0"]
                layer_mm("Wself0", st[u]["h0"], psR0, start=False)
                del st[u]["h0"]
                st[u]["psR0"] = psR0

            # [Act] enc-evac(s)
            if 0 <= s < IT:
                h0 = pool.tile([128, GRP], BF16, tag="h0", bufs=3,
                               name=f"h0_{s}")
                nc.scalar.activation(h0[:], st[s]["psE"][:], Relu,
                                     bias=BIAS["be"][:])
                st[s]["h0"] = h0
                del st[s]["psE"]

            # [Act] r0-evac(s-1)
            u = s - 1
            if 0 <= u < IT:
                h1 = pool.tile([128, GRP], BF16, tag="h1", bufs=3,
                               name=f"h1_{u}")
                nc.scalar.activation(h1[:], st[u]["psR0"][:], Relu)
                st[u]["h1"] = h1
                del st[u]["psR0"]

            # [PE] round-0 T-pass (bias folded): psT0(s) from h0(s)
            if 0 <= s < IT:
                psT0 = psum.tile([128, NS], F32, tag="psT0", bufs=1,
                                 name=f"psT0_{s}")
                h0 = st[s]["h0"]
                for g in range(2):
                    sl = slice(64 * g, 64 * g + 64)
                    rhs = h0[sl, :].rearrange("p (S n) -> p n S", n=N)
                    outT = psT0[sl, :].unsqueeze(1).broadcast_to([64, N, NS])
                    nc.tensor.matmul(outT, W["Wsum0"][sl, :], rhs,
                                     start=True, stop=False)
                    nc.tensor.matmul(psT0[sl, :],
                                     W["bm0"][64 * g:64 * g + 1, :],
                                     ones16[64 * g:64 * g + 1, :],
                                     start=False, stop=True)
                st[s]["psT0"] = psT0

            # [Act] preload T1(s-3) into psR1; [PE] s1; [DVE] r1-evac
            u = s - 3
            if 0 <= u < IT:
                psR1 = big("R1", u)
                nc.scalar.activation(
                    psR1[:].rearrange("p (S n) -> p S n", n=N),
                    st[u]["psT1"][:].unsqueeze(2).broadcast_to([128, NS, N]),
                    Copy)
                del st[u]["psT1"]
                layer_mm("Wself1", st[u]["h1"], psR1, start=False)
                del st[u]["h1"]
                h2 = pool.tile([128, GRP], BF16, tag="h2", bufs=2,
                               name=f"h2_{u}")
                nc.vector.tensor_scalar(h2[:], psR1[:], 0.0, None, ALU.max)
                st[u]["h2"] = h2

            # [DVE] S1(s-2); [PE] psT1(s-2) = Wsum1^T S1 + b1
            u = s - 2
            if 0 <= u < IT:
                S1 = pool.tile([128, NS], BF16, tag="S1", bufs=2,
                               name=f"S1_{u}")
                with nc.allow_low_precision("agent-sum in bf16"):
                    nc.vector.tensor_reduce(
                        S1[:], st[u]["h1"][:].rearrange("p (S n) -> p S n", n=N),
                        mybir.AxisListType.X, ALU.add)
                psT1 = psum.tile([128, NS], F32, tag="psT1", bufs=1,
                                 name=f"psT1_{u}")
                for g in range(2):
                    sl = slice(64 * g, 64 * g + 64)
                    nc.tensor.matmul(psT1[sl, :], W["Wsum1"][sl, :],
                                     S1[sl, :], start=True, stop=False)
                    nc.tensor.matmul(psT1[sl, :],
                                     W["bm1"][64 * g:64 * g + 1, :],
                                     ones16[64 * g:64 * g + 1, :],
                                     start=False, stop=True)
                st[u]["psT1"] = psT1

            # [PE] out1(s-4); [DVE] hid-evac(s-4)
            u = s - 4
            if 0 <= u < IT:
                psH = layer_mm("W1", st[u]["h2"], big("H", u))
                del st[u]["h2"]
                hid = pool.tile([128, GRP], BF16, tag="hid", bufs=2,
                                name=f"hid{u}")
                nc.vector.tensor_scalar(hid[:], psH[:], BIAS["bh"][:], 0.0,
                                        ALU.add, ALU.max)
                st[u]["hid"] = hid

            # [PE] W2(s-5); [DVE] q-evac + [SP] store at quad end
            u = s - 5
            if 0 <= u < IT:
                k = u % 4
                if k == 0:
                    st[u]["psQ"] = psum.tile([128, GRP], F32, tag="psQ",
                                             bufs=1, name=f"psQ{u}")
                else:
                    st[u]["psQ"] = st[u - 1].pop("psQ")
                psQ = st[u]["psQ"]
                nc.tensor.matmul(psQ[32 * k:32 * k + 32, :], W["W2"][:],
                                 st[u]["hid"][:], start=True, stop=True,
                                 tile_position=(0, 32 * k),
                                 skip_group_check=True)
                del st[u]["hid"]
                if k == 3:
                    q_sb = pool.tile([128, GRP], BF16, tag="qsb", bufs=2,
                                     name=f"qsb{u}")
                    nc.vector.tensor_tensor(q_sb[:], psQ[:],
                                            st[u - 3]["pen"][:], ALU.add)
                    nc.sync.dma_start(q_d[u // 4], q_sb[:])
                    del st[u]["psQ"], st[u - 3]["pen"]

    nc.compile()
    return nc


def _prep_host(obs, enc_w, enc_b, comm_w, comm_b, out_w1, out_b1, out_w2, out_b2,
               available_actions):
    """Build per-core input maps (packed layouts + derived weights)."""
    bf16 = ml_dtypes.bfloat16
    f32 = np.float32

    def rep(w):  # replicate [64, m] weight onto both partition halves
        return np.ascontiguousarray(np.concatenate([w, w], axis=0)
                                    .astype(f32)).astype(bf16)

    def bd(w):  # block-diag duplicate [k,m] -> [2k, 2m]
        k, m = w.shape
        o = np.zeros((2 * k, 2 * m), f32)
        o[:k, :m] = w
        o[k:, m:] = w
        return np.ascontiguousarray(o).astype(bf16)

    def brow(b):  # bias as a [128, 64] tile with every row = b
        return np.ascontiguousarray(np.tile(b.astype(f32)[None, :], (128, 1))
                                    ).astype(bf16)

    wm0 = comm_w[0][H:].astype(f32) / (N - 1)
    wm1 = comm_w[1][H:].astype(f32) / (N - 1)
    weights = {
        "Wenc": rep(enc_w),
        "Wself0": rep(comm_w[0][:H].astype(f32) - wm0),
        "Wself1": rep(comm_w[1][:H].astype(f32) - wm1),
        "Wsum0": rep(wm0),
        "Wsum1": rep(wm1),
        "W1": rep(out_w1),
        "W2": bd(out_w2),
        "bm0": brow(comm_b[0]),
        "bm1": brow(comm_b[1]),
    }
    biases = {"be": enc_b, "bh": out_b1}
    biases = {k: np.concatenate([v, v]).astype(f32).reshape(128, 1)
              for k, v in biases.items()}

    rows = np.ascontiguousarray(obs.reshape(B * N, OBS))
    pen = np.where(available_actions.reshape(B * N, A) == 0,
                   f32(-1e10), out_b2.astype(f32)[None, :]).astype(bf16)

    in_maps = []
    for c in range(NCORES):
        ro = rows[c * RPC:(c + 1) * RPC]
        # [IT, group, row, feat] -> [IT, group*feat, row]
        opk = ro.reshape(IT, 2, GRP, OBS).transpose(0, 1, 3, 2) \
                .reshape(IT, 128, GRP).astype(bf16)
        pe = pen[c * RPC:(c + 1) * RPC]
        # pen/q partitions: [iter%4, group, action]
        ppk = pe.reshape(IT // 4, 4, 2, GRP, A).transpose(0, 1, 2, 4, 3) \
                .reshape(IT // 4, 128, GRP)
        m = {"obs_pk": np.ascontiguousarray(opk),
             "pen_pk": np.ascontiguousarray(ppk)}
        m.update(weights)
        m.update(biases)
        in_maps.append(m)
    return in_maps


def _unpack_output(results):
    qs = []
    for r in results:
        qpk = np.asarray(r["q_pk"]).astype(np.float32)  # [IT//4, 128, GRP]
        q = qpk.reshape(IT // 4, 4, 2, A, GRP).transpose(0, 1, 2, 4, 3) \
               .reshape(RPC, A)
        qs.append(q)
    return np.concatenate(qs, axis=0).reshape(B, N, A)


def run_on_device(in_maps, trace=False):
    from concourse.bass_utils import run_bass_kernel_spmd

    if "nc" not in _cache:
        _cache["nc"] = _build_device_program()
    return run_bass_kernel_spmd(_cache["nc"], in_maps,
                                core_ids=list(range(NCORES)), trace=trace)


def kernel(obs, enc_w, enc_b, comm_w, comm_b, out_w1, out_b1, out_w2, out_b2,
           available_actions):
    args = [np.asarray(x) for x in
            (obs, enc_w, enc_b, comm_w, comm_b, out_w1, out_b1, out_w2, out_b2,
             available_actions)]
    in_maps = _prep_host(*args)
    res = run_on_device(in_maps)
    return _unpack_output(res.results)
